# revision 24
# baseline (speedup 1.0000x reference)
"""Causal self-attention on 8 TRN2 NeuronCores (Bass/Tile, fp32r).

Sharding: core c = 4*bp + hg handles batches [2bp, 2bp+1] and heads
[4hg, 4hg+4). Host transposes x to [B, D, S], slices weights per head
group, sums the 4 head-group partial outputs per batch pair.

Per-core kernel (per batch):
  A) QKV projection from xT tiles: qT/kT in [head_dim, token] layout
     (matmul lhsT = w slice), v directly in [token, head_dim] layout
     (matmul lhsT = xT tile, rhs = w_v) with a ones column appended.
  B) Flash-style causal attention per head: scores^T blocks [j,i] on PE,
     exp (with 1/sqrt(hd) scale folded in) on ACT, mask on diagonal
     blocks on DVE, AV accumulation on PE; ones column of V yields the
     softmax denominators in psum row 64; normalize via reciprocal +
     partition-broadcast DMA + DVE multiply.
  C) Output projection (partial: only this core's head rows of w_proj).
"""
import numpy as np

B, S, D, H = 4, 2048, 1024, 16
HD = D // H            # 64
SCALE = 1.0 / np.sqrt(HD)
NB = 2                 # batches per core
NHC = 4                # heads per core
HCOLS = NHC * HD       # 256 q/k/v columns per core
NDT = D // 128         # 8 D-tiles
NJT = S // 128         # 16 j-tiles per batch
NIT = S // 512         # 4 i-tiles per batch

_NC = None
LAST_RESULT = None


def _build():
    import concourse.bacc as bacc
    import concourse.mybir as mybir
    import concourse.tile as tile

    f32 = mybir.dt.float32
    f32r = mybir.dt.float32r
    Act = mybir.ActivationFunctionType

    nc = bacc.Bacc(trn_type="TRN2", target_bir_lowering=False)
    xT = nc.dram_tensor("xT", [NB, D, S], f32, kind="ExternalInput")
    wqk = nc.dram_tensor("wqk", [D, 2 * HCOLS], f32, kind="ExternalInput")
    wv = nc.dram_tensor("wv", [D, HCOLS], f32, kind="ExternalInput")
    wo = nc.dram_tensor("wo", [HCOLS, D], f32, kind="ExternalInput")
    bqk = nc.dram_tensor("bqk", [2 * HCOLS], f32, kind="ExternalInput")
    bv = nc.dram_tensor("bv", [HCOLS], f32, kind="ExternalInput")
    mask = nc.dram_tensor("mask", [128, 4 * 512], f32, kind="ExternalInput")
    y = nc.dram_tensor("y", [NB, S, D], f32, kind="ExternalOutput")

    import concourse.bass as bass

    with tile.TileContext(nc) as tc:
        with (
            tc.tile_pool(name="singles", bufs=1) as singles,
            tc.tile_pool(name="xtp", bufs=2) as xtp,
            tc.tile_pool(name="qkp", bufs=1) as qkp,
            tc.tile_pool(name="vp", bufs=1) as vp,
            tc.tile_pool(name="attp", bufs=5) as attp,
            tc.tile_pool(name="yhp", bufs=1) as yhp,
            tc.tile_pool(name="stgp", bufs=6) as stgp,
            tc.tile_pool(name="bcp", bufs=1) as bcp,
            tc.tile_pool(name="outp", bufs=2) as outp,
            tc.tile_pool(name="dscr", bufs=2, space="DRAM") as dscrp,
            tc.tile_pool(name="psX", bufs=2, space="PSUM") as psX,
            tc.tile_pool(name="psY", bufs=4, space="PSUM") as psY,
        ):
            # ---- one-time loads (weights, biases, masks) ----
            wqk_sb = singles.tile([128, NDT, 2 * HCOLS], f32r, tag="wqk")
            nc.sync.dma_start(
                out=wqk_sb,
                in_=wqk.ap().bitcast(f32r).rearrange("(dt p) c -> p dt c", p=128),
            )
            wv_sb = singles.tile([128, NDT, HCOLS], f32r, tag="wv")
            nc.sync.dma_start(
                out=wv_sb,
                in_=wv.ap().bitcast(f32r).rearrange("(dt p) c -> p dt c", p=128),
            )
            wo_sb = singles.tile([128, 2, D], f32r, tag="wo")
            nc.sync.dma_start(
                out=wo_sb,
                in_=wo.ap().bitcast(f32r).rearrange("(kt p) c -> p kt c", p=128),
            )
            bqk_sb = singles.tile([128, 4], f32, tag="bqk")
            nc.sync.dma_start(
                out=bqk_sb, in_=bqk.ap().rearrange("(cb p) -> p cb", p=128)
            )
            # bv broadcast across partitions: [HCOLS] -> [128, HCOLS]
            bv_sb = singles.tile([128, HCOLS], f32, tag="bv")
            bv_ap = bv.ap()
            nc.gpsimd.dma_start(
                out=bv_sb,
                in_=bass.AP(
                    tensor=bv_ap.tensor, offset=bv_ap.offset,
                    ap=[[0, 128], *bv_ap.ap],
                ),
            )
            ones_sb = singles.tile([128, 64], f32, tag="ones")
            nc.vector.memset(ones_sb[:], 1.0)
            zeros_sb = singles.tile([128, 1024], f32, tag="zeros")
            nc.vector.memset(zeros_sb[:], 0.0)
            mask_sb = singles.tile([128, 4, 512], f32r, tag="mask")
            nc.sync.dma_start(
                out=mask_sb,
                in_=mask.ap().bitcast(f32r).rearrange("p (r i) -> p r i", r=4),
            )

            for b in range(NB):
                # ================= Phase A: QKV projection =================
                sA = nc.enter_named_scope(f"qkv{b}", False)
                # q: per-head zero-padded [128, S] tiles (rows 64*(h%2) hold
                # q_h, other 64 rows zero) so scores matmuls run K=128 with the
                # packed kT tile as weights -- K<128 matmuls never warm the PE
                # clock gate. k: packed [2 tiles of 128 rows, S].
                qp = [
                    qkp.tile([128, S], f32r, tag=f"qp{h}", name=f"qp{h}_{b}")
                    for h in range(NHC)
                ]
                kT = [
                    qkp.tile([128, S], f32r, tag=f"kT{g}", name=f"kT{g}_{b}")
                    for g in range(2)
                ]
                if b == 0:  # zero the pad halves once (bufs=1 slots persist)
                    for h in range(NHC):
                        zo = 64 * ((h + 1) % 2)
                        for half in range(2):
                            nc.vector.tensor_copy(
                                qp[h][zo : zo + 64,
                                      half * 1024 : (half + 1) * 1024],
                                zeros_sb[0:64, :],
                            )
                v_sb = vp.tile([128, NJT, NHC, HD + 1], f32r, tag="v")
                nc.vector.tensor_copy(
                    v_sb[:, :, :, HD : HD + 1],
                    ones_sb[:].rearrange("p (a b c) -> p a b c", a=NJT, b=NHC),
                )

                for ch in range(4):  # 512-token chunks
                    t0 = ch * 512
                    xt = xtp.tile([128, NDT, 512], f32r, tag="xt")
                    for dt in range(NDT):
                        nc.sync.dma_start(
                            out=xt[:, dt, :],
                            in_=xT.ap().bitcast(f32r)[
                                b, dt * 128 : (dt + 1) * 128, t0 : t0 + 512
                            ],
                        )
                    for cb in range(4):  # q0 q1 k0 k1
                        psw = psX.tile([128, 1024], f32, tag="sc",
                                       name=f"psqk_{b}_{ch}_{cb}")
                        ps = psw[:, 0:512]
                        for dt in range(NDT):
                            nc.tensor.matmul(
                                ps,
                                wqk_sb[:, dt, cb * 128 : (cb + 1) * 128],
                                xt[:, dt, :],
                                start=(dt == 0), stop=(dt == NDT - 1),
                            )
                        if cb < 2:  # q tiles: split halves into padded tiles
                            for hh in range(2):
                                po2 = 64 * hh
                                nc.vector.tensor_scalar_add(
                                    out=qp[2 * cb + hh][po2 : po2 + 64,
                                                        t0 : t0 + 512],
                                    in0=psw[po2 : po2 + 64, 0:512],
                                    scalar1=bqk_sb[po2 : po2 + 64, cb : cb + 1],
                                )
                        else:
                            nc.vector.tensor_scalar_add(
                                out=kT[cb - 2][:, t0 : t0 + 512],
                                in0=ps,
                                scalar1=bqk_sb[:, cb : cb + 1],
                            )
                    for st in range(4):  # 128-token tiles within chunk
                        tok = t0 + st * 128
                        psv = psX.tile(
                            [128, 1024], f32, tag="sc", name=f"psv_{b}_{tok}"
                        )
                        for dt in range(NDT):
                            nc.tensor.matmul(
                                psv[:, 0:HCOLS],
                                xt[:, dt, st * 128 : (st + 1) * 128],
                                wv_sb[:, dt, :],
                                start=(dt == 0), stop=(dt == NDT - 1),
                            )
                        nc.vector.tensor_add(
                            v_sb[:, tok // 128, :, 0:HD],
                            psv[:, 0:HCOLS].rearrange("p (h c) -> p h c", h=NHC),
                            bv_sb[:].rearrange("p (h c) -> p h c", h=NHC),
                        )

                nc.leave_named_scope(f"qkv{b}", sA[0], False)
                # ================= Phase B: causal attention =================
                sB = nc.enter_named_scope(f"attn{b}", False)
                yh = [
                    yhp.tile([128, S], f32r, tag=f"yh{g}", name=f"yh{g}_{b}")
                    for g in range(2)
                ]

                for h in range(NHC):
                    g, po = h // 2, 64 * (h % 2)
                    # jt-major: each kT j-tile / V j-tile weight is used by
                    # up to 4 consecutive matmuls (one per live i-tile), so
                    # the expensive fp32r weight load amortizes. All 4 AV
                    # accumulators stay live (psY bufs=4).
                    psy = [
                        psY.tile([HD + 1, 512], f32, tag="psy",
                                 name=f"psy_{b}_{h}_{it}")
                        for it in range(NIT)
                    ]
                    atts = {}
                    for jt in range(NJT):
                        it_lo = jt // 4
                        # group i-tiles in pairs sharing one 2-bank psum tile
                        # and ONE exp over [128, 1024]
                        groups = []
                        k0 = it_lo
                        while k0 < NIT:
                            w = 2 if k0 + 1 < NIT else 1
                            groups.append((k0, w))
                            k0 += w
                        for (i0, w) in groups:
                            pss = psX.tile(
                                [128, 1024], f32, tag="sc",
                                name=f"pss_{b}_{h}_{i0}_{jt}",
                            )
                            for u in range(w):
                                nc.tensor.matmul(
                                    pss[:, u * 512 : (u + 1) * 512],
                                    kT[g][:, jt * 128 : (jt + 1) * 128],
                                    qp[h][:, (i0 + u) * 512 : (i0 + u + 1) * 512],
                                    start=True, stop=True,
                                )
                            att = attp.tile([128, 1024], f32r, tag="att")
                            nc.scalar.activation(
                                out=att[:, 0 : w * 512],
                                in_=pss[:, 0 : w * 512], func=Act.Exp,
                                bias=0.0, scale=float(SCALE),
                            )
                            if i0 == it_lo and jt - 4 * it_lo < 4:
                                # diagonal block: causal mask (split engines)
                                eng = nc.vector if jt % 2 else nc.gpsimd
                                eng.tensor_mul(
                                    att[:, 0:512], att[:, 0:512],
                                    mask_sb[:, jt - 4 * it_lo, :],
                                )
                            for u in range(w):
                                atts[i0 + u] = att[:, u * 512 : (u + 1) * 512]
                        for it in range(it_lo, NIT):
                            nc.tensor.matmul(
                                psy[it][:],
                                v_sb[:, jt, h, :],
                                atts[it],
                                start=(jt == 0), stop=(jt == 4 * it + 3),
                            )
                    # stage AV results out of PSUM fast (frees accumulator
                    # slots); denominator rows go to a DRAM scratch, packed
                    # across 128 partitions for a cheap reciprocal, scattered
                    # back to DRAM, broadcast-read, and multiplied in.
                    dh = dscrp.tile([NIT, 512], f32, tag="dh",
                                    name=f"dh_{b}_{h}")
                    drec = dscrp.tile([NIT * 512], f32, tag="drec",
                                      name=f"drec_{b}_{h}")
                    hstgs = []
                    for it in range(NIT):
                        stg = stgp.tile([HD + 1, 512], f32, tag="stg",
                                        name=f"stg_{b}_{h}_{it}")
                        nc.scalar.copy(stg[:], psy[it][:])
                        hstgs.append(stg)
                        nc.sync.dma_start(out=dh[it, :], in_=stg[64:65, :])
                    pk = stgp.tile([128, 16], f32, tag="pk",
                                   name=f"pk_{b}_{h}")
                    nc.sync.dma_start(
                        out=pk[:],
                        in_=bass.AP(tensor=dh.tensor, offset=dh.offset,
                                    ap=[[16, 128], [1, 16]]),
                    )
                    nc.vector.reciprocal(pk[:], pk[:])
                    nc.sync.dma_start(
                        out=bass.AP(tensor=drec.tensor, offset=drec.offset,
                                    ap=[[16, 128], [1, 16]]),
                        in_=pk[:],
                    )
                    bch = bcp.tile([64, NIT, 512], f32, tag="bc4",
                                   name=f"bc4_{b}_{h}")
                    nc.gpsimd.dma_start(
                        out=bch[:],
                        in_=bass.AP(
                            tensor=drec.tensor, offset=drec.offset,
                            ap=[[0, 64], [1, NIT * 512]],
                        ),
                    )
                    for it in range(NIT):
                        nc.vector.tensor_mul(
                            yh[g][po : po + 64, it * 512 : (it + 1) * 512],
                            hstgs[it][0:HD, :], bch[:, it, :],
                        )

                nc.leave_named_scope(f"attn{b}", sB[0], False)
                # ================= Phase C: output projection =================
                sC = nc.enter_named_scope(f"proj{b}", False)
                for tt2 in range(S // 128):
                    yo = outp.tile([128, D], f32, tag="yo")
                    for oc in range(2):
                        psow = psX.tile(
                            [128, 1024], f32, tag="sc", name=f"pso_{b}_{tt2}_{oc}"
                        )
                        pso = psow[:, 0:512]
                        for kt in range(2):
                            nc.tensor.matmul(
                                pso,
                                yh[kt][:, tt2 * 128 : (tt2 + 1) * 128],
                                wo_sb[:, kt, oc * 512 : (oc + 1) * 512],
                                start=(kt == 0), stop=(kt == 1),
                            )
                        nc.vector.tensor_copy(yo[:, oc * 512 : (oc + 1) * 512], pso)
                    nc.sync.dma_start(
                        out=y.ap()[b, tt2 * 128 : (tt2 + 1) * 128, :], in_=yo[:]
                    )

                nc.leave_named_scope(f"proj{b}", sC[0], False)

    nc.compile()
    return nc


def _get_nc():
    global _NC
    if _NC is None:
        _NC = _build()
    return _NC


def kernel(x, w_qkv, b_qkv, w_proj, b_proj):
    global LAST_RESULT
    from concourse.bass_utils import run_bass_kernel_spmd

    x = np.asarray(x, dtype=np.float32)
    w_qkv = np.asarray(w_qkv, dtype=np.float32)
    b_qkv = np.asarray(b_qkv, dtype=np.float32)
    w_proj = np.asarray(w_proj, dtype=np.float32)
    b_proj = np.asarray(b_proj, dtype=np.float32)

    xTb = np.ascontiguousarray(x.transpose(0, 2, 1))  # [B, D, S]

    # causal masks for the 4 diagonal block offsets r: allow j'+128r <= i'
    jj = np.arange(128)[:, None]
    ii = np.arange(512)[None, :]
    mask = np.concatenate(
        [(jj + 128 * r <= ii).astype(np.float32) for r in range(4)], axis=1
    )  # [128, 2048]

    in_maps = []
    for c in range(8):
        bp, hg = c // 4, c % 4
        cols = slice(hg * HCOLS, (hg + 1) * HCOLS)
        w_q = w_qkv[:, cols]
        w_k = w_qkv[:, D : 2 * D][:, cols]
        w_v = w_qkv[:, 2 * D : 3 * D][:, cols]
        in_maps.append({
            "xT": np.ascontiguousarray(xTb[2 * bp : 2 * bp + 2]),
            "wqk": np.ascontiguousarray(np.concatenate([w_q, w_k], axis=1)),
            "wv": np.ascontiguousarray(w_v),
            "wo": np.ascontiguousarray(w_proj[cols, :]),
            "bqk": np.ascontiguousarray(
                np.concatenate([b_qkv[cols], b_qkv[D : 2 * D][cols]])
            ),
            "bv": np.ascontiguousarray(b_qkv[2 * D : 3 * D][cols]),
            "mask": mask,
        })

    nc = _get_nc()
    res = run_bass_kernel_spmd(nc, in_maps, core_ids=list(range(8)))
    LAST_RESULT = res

    out = np.zeros((B, S, D), dtype=np.float32)
    for c in range(8):
        bp = c // 4
        out[2 * bp : 2 * bp + 2] += res.results[c]["y"]
    out += b_proj[None, None, :]
    return out


# revision 25
# speedup vs baseline: 1.0421x; 1.0421x over previous
"""Causal self-attention on 8 TRN2 NeuronCores (Bass/Tile, fp32r).

Sharding: core c = 4*bp + hg handles batches [2bp, 2bp+1] and heads
[4hg, 4hg+4). Host transposes x to [B, D, S], slices weights per head
group, sums the 4 head-group partial outputs per batch pair.

Per-core kernel (per batch):
  A) QKV projection from xT tiles: qT/kT in [head_dim, token] layout
     (matmul lhsT = w slice), v directly in [token, head_dim] layout
     (matmul lhsT = xT tile, rhs = w_v) with a ones column appended.
  B) Flash-style causal attention per head: scores^T blocks [j,i] on PE,
     exp (with 1/sqrt(hd) scale folded in) on ACT, mask on diagonal
     blocks on DVE, AV accumulation on PE; ones column of V yields the
     softmax denominators in psum row 64; normalize via reciprocal +
     partition-broadcast DMA + DVE multiply.
  C) Output projection (partial: only this core's head rows of w_proj).
"""
import numpy as np

B, S, D, H = 4, 2048, 1024, 16
HD = D // H            # 64
SCALE = 1.0 / np.sqrt(HD)
NB = 2                 # batches per core
NHC = 4                # heads per core
HCOLS = NHC * HD       # 256 q/k/v columns per core
NDT = D // 128         # 8 D-tiles
NJT = S // 128         # 16 j-tiles per batch
NIT = S // 512         # 4 i-tiles per batch

_NC = None
LAST_RESULT = None


def _build():
    import concourse.bacc as bacc
    import concourse.mybir as mybir
    import concourse.tile as tile

    f32 = mybir.dt.float32
    f32r = mybir.dt.float32r
    Act = mybir.ActivationFunctionType

    nc = bacc.Bacc(trn_type="TRN2", target_bir_lowering=False)
    xT = nc.dram_tensor("xT", [NB, D, S], f32, kind="ExternalInput")
    wqk = nc.dram_tensor("wqk", [D, 2 * HCOLS], f32, kind="ExternalInput")
    wv = nc.dram_tensor("wv", [D, HCOLS], f32, kind="ExternalInput")
    wo = nc.dram_tensor("wo", [HCOLS, D], f32, kind="ExternalInput")
    bqk = nc.dram_tensor("bqk", [2 * HCOLS], f32, kind="ExternalInput")
    bv = nc.dram_tensor("bv", [HCOLS], f32, kind="ExternalInput")
    mask = nc.dram_tensor("mask", [128, 4 * 512], f32, kind="ExternalInput")
    y = nc.dram_tensor("y", [NB, S, D], f32, kind="ExternalOutput")

    import concourse.bass as bass

    with tile.TileContext(nc) as tc:
        with (
            tc.tile_pool(name="singles", bufs=1) as singles,
            tc.tile_pool(name="xtp", bufs=2) as xtp,
            tc.tile_pool(name="qkp", bufs=1) as qkp,
            tc.tile_pool(name="vp", bufs=1) as vp,
            tc.tile_pool(name="attp", bufs=5) as attp,
            tc.tile_pool(name="yhp", bufs=1) as yhp,
            tc.tile_pool(name="stgp", bufs=6) as stgp,
            tc.tile_pool(name="bcp", bufs=1) as bcp,
            tc.tile_pool(name="outp", bufs=2) as outp,
            tc.tile_pool(name="dscr", bufs=2, space="DRAM") as dscrp,
            tc.tile_pool(name="psX", bufs=2, space="PSUM") as psX,
            tc.tile_pool(name="psY", bufs=4, space="PSUM") as psY,
        ):
            # ---- one-time loads (weights, biases, masks) ----
            wqk_sb = singles.tile([128, NDT, 2 * HCOLS], f32r, tag="wqk")
            nc.sync.dma_start(
                out=wqk_sb,
                in_=wqk.ap().bitcast(f32r).rearrange("(dt p) c -> p dt c", p=128),
            )
            wv_sb = singles.tile([128, NDT, HCOLS], f32r, tag="wv")
            nc.sync.dma_start(
                out=wv_sb,
                in_=wv.ap().bitcast(f32r).rearrange("(dt p) c -> p dt c", p=128),
            )
            wo_sb = singles.tile([128, 2, D], f32r, tag="wo")
            nc.sync.dma_start(
                out=wo_sb,
                in_=wo.ap().bitcast(f32r).rearrange("(kt p) c -> p kt c", p=128),
            )
            bqk_sb = singles.tile([128, 4], f32, tag="bqk")
            nc.sync.dma_start(
                out=bqk_sb, in_=bqk.ap().rearrange("(cb p) -> p cb", p=128)
            )
            # bv broadcast across partitions: [HCOLS] -> [128, HCOLS]
            bv_sb = singles.tile([128, HCOLS], f32, tag="bv")
            bv_ap = bv.ap()
            nc.gpsimd.dma_start(
                out=bv_sb,
                in_=bass.AP(
                    tensor=bv_ap.tensor, offset=bv_ap.offset,
                    ap=[[0, 128], *bv_ap.ap],
                ),
            )
            ones_sb = singles.tile([128, 64], f32, tag="ones")
            nc.vector.memset(ones_sb[:], 1.0)
            zeros_sb = singles.tile([128, 1024], f32, tag="zeros")
            nc.vector.memset(zeros_sb[:], 0.0)
            mask_sb = singles.tile([128, 4, 512], f32r, tag="mask")
            nc.sync.dma_start(
                out=mask_sb,
                in_=mask.ap().bitcast(f32r).rearrange("p (r i) -> p r i", r=4),
            )

            for b in range(NB):
                # ================= Phase A: QKV projection =================
                sA = nc.enter_named_scope(f"qkv{b}", False)
                # q: per-head zero-padded [128, S] tiles (rows 64*(h%2) hold
                # q_h, other 64 rows zero) so scores matmuls run K=128 with the
                # packed kT tile as weights -- K<128 matmuls never warm the PE
                # clock gate. k: packed [2 tiles of 128 rows, S].
                qp = [
                    qkp.tile([128, S], f32r, tag=f"qp{h}", name=f"qp{h}_{b}")
                    for h in range(NHC)
                ]
                kT = [
                    qkp.tile([128, S], f32r, tag=f"kT{g}", name=f"kT{g}_{b}")
                    for g in range(2)
                ]
                if b == 0:  # zero the pad halves once (bufs=1 slots persist)
                    for h in range(NHC):
                        zo = 64 * ((h + 1) % 2)
                        for half in range(2):
                            nc.vector.tensor_copy(
                                qp[h][zo : zo + 64,
                                      half * 1024 : (half + 1) * 1024],
                                zeros_sb[0:64, :],
                            )
                v_sb = vp.tile([128, NJT, NHC, HD + 1], f32r, tag="v")
                nc.vector.tensor_copy(
                    v_sb[:, :, :, HD : HD + 1],
                    ones_sb[:].rearrange("p (a b c) -> p a b c", a=NJT, b=NHC),
                )

                for ch in range(4):  # 512-token chunks
                    t0 = ch * 512
                    xt = xtp.tile([128, NDT, 512], f32r, tag="xt")
                    for dt in range(NDT):
                        nc.sync.dma_start(
                            out=xt[:, dt, :],
                            in_=xT.ap().bitcast(f32r)[
                                b, dt * 128 : (dt + 1) * 128, t0 : t0 + 512
                            ],
                        )
                    for cb in range(4):  # q0 q1 k0 k1
                        psw = psY.tile([128, 512], f32, tag="psy",
                                       name=f"psqk_{b}_{ch}_{cb}")
                        ps = psw[:, 0:512]
                        for dt in range(NDT):
                            nc.tensor.matmul(
                                ps,
                                wqk_sb[:, dt, cb * 128 : (cb + 1) * 128],
                                xt[:, dt, :],
                                start=(dt == 0), stop=(dt == NDT - 1),
                            )
                        if cb < 2:  # q tiles: split halves into padded tiles
                            for hh in range(2):
                                po2 = 64 * hh
                                nc.vector.tensor_scalar_add(
                                    out=qp[2 * cb + hh][po2 : po2 + 64,
                                                        t0 : t0 + 512],
                                    in0=psw[po2 : po2 + 64, 0:512],
                                    scalar1=bqk_sb[po2 : po2 + 64, cb : cb + 1],
                                )
                        else:
                            nc.vector.tensor_scalar_add(
                                out=kT[cb - 2][:, t0 : t0 + 512],
                                in0=ps,
                                scalar1=bqk_sb[:, cb : cb + 1],
                            )
                    for st in range(4):  # 128-token tiles within chunk
                        tok = t0 + st * 128
                        psv = psX.tile(
                            [128, 1024], f32, tag="sc", name=f"psv_{b}_{tok}"
                        )
                        for dt in range(NDT):
                            nc.tensor.matmul(
                                psv[:, 0:HCOLS],
                                xt[:, dt, st * 128 : (st + 1) * 128],
                                wv_sb[:, dt, :],
                                start=(dt == 0), stop=(dt == NDT - 1),
                            )
                        nc.vector.tensor_add(
                            v_sb[:, tok // 128, :, 0:HD],
                            psv[:, 0:HCOLS].rearrange("p (h c) -> p h c", h=NHC),
                            bv_sb[:].rearrange("p (h c) -> p h c", h=NHC),
                        )

                nc.leave_named_scope(f"qkv{b}", sA[0], False)
                # ================= Phase B: causal attention =================
                sB = nc.enter_named_scope(f"attn{b}", False)
                yh = [
                    yhp.tile([128, S], f32r, tag=f"yh{g}", name=f"yh{g}_{b}")
                    for g in range(2)
                ]

                for h in range(NHC):
                    g, po = h // 2, 64 * (h % 2)
                    # jt-major: each kT j-tile / V j-tile weight is used by
                    # up to 4 consecutive matmuls (one per live i-tile), so
                    # the expensive fp32r weight load amortizes. All 4 AV
                    # accumulators stay live (psY bufs=4).
                    psy = [
                        psY.tile([HD + 1, 512], f32, tag="psy",
                                 name=f"psy_{b}_{h}_{it}")
                        for it in range(NIT)
                    ]
                    atts = {}
                    for jt in range(NJT):
                        it_lo = jt // 4
                        # group i-tiles in pairs sharing one 2-bank psum tile
                        # and ONE exp over [128, 1024]
                        groups = []
                        k0 = it_lo
                        while k0 < NIT:
                            w = 2 if k0 + 1 < NIT else 1
                            groups.append((k0, w))
                            k0 += w
                        for (i0, w) in groups:
                            pss = psX.tile(
                                [128, 1024], f32, tag="sc",
                                name=f"pss_{b}_{h}_{i0}_{jt}",
                            )
                            for u in range(w):
                                nc.tensor.matmul(
                                    pss[:, u * 512 : (u + 1) * 512],
                                    kT[g][:, jt * 128 : (jt + 1) * 128],
                                    qp[h][:, (i0 + u) * 512 : (i0 + u + 1) * 512],
                                    start=True, stop=True,
                                )
                            att = attp.tile([128, 1024], f32r, tag="att")
                            nc.scalar.activation(
                                out=att[:, 0 : w * 512],
                                in_=pss[:, 0 : w * 512], func=Act.Exp,
                                bias=0.0, scale=float(SCALE),
                            )
                            if i0 == it_lo and jt - 4 * it_lo < 4:
                                # diagonal block: causal mask (split engines)
                                eng = nc.vector if jt % 2 else nc.gpsimd
                                eng.tensor_mul(
                                    att[:, 0:512], att[:, 0:512],
                                    mask_sb[:, jt - 4 * it_lo, :],
                                )
                            for u in range(w):
                                atts[i0 + u] = att[:, u * 512 : (u + 1) * 512]
                        for it in range(it_lo, NIT):
                            nc.tensor.matmul(
                                psy[it][:],
                                v_sb[:, jt, h, :],
                                atts[it],
                                start=(jt == 0), stop=(jt == 4 * it + 3),
                            )
                    # stage AV results out of PSUM fast (frees accumulator
                    # slots); denominator rows go to a DRAM scratch, packed
                    # across 128 partitions for a cheap reciprocal, scattered
                    # back to DRAM, broadcast-read, and multiplied in.
                    dh = dscrp.tile([NIT, 512], f32, tag="dh",
                                    name=f"dh_{b}_{h}")
                    drec = dscrp.tile([NIT * 512], f32, tag="drec",
                                      name=f"drec_{b}_{h}")
                    hstgs = []
                    for it in range(NIT):
                        stg = stgp.tile([HD + 1, 512], f32, tag="stg",
                                        name=f"stg_{b}_{h}_{it}")
                        nc.scalar.copy(stg[:], psy[it][:])
                        hstgs.append(stg)
                        nc.sync.dma_start(out=dh[it, :], in_=stg[64:65, :])
                    pk = stgp.tile([128, 16], f32, tag="pk",
                                   name=f"pk_{b}_{h}")
                    nc.sync.dma_start(
                        out=pk[:],
                        in_=bass.AP(tensor=dh.tensor, offset=dh.offset,
                                    ap=[[16, 128], [1, 16]]),
                    )
                    nc.vector.reciprocal(pk[:], pk[:])
                    nc.sync.dma_start(
                        out=bass.AP(tensor=drec.tensor, offset=drec.offset,
                                    ap=[[16, 128], [1, 16]]),
                        in_=pk[:],
                    )
                    bch = bcp.tile([64, NIT, 512], f32, tag="bc4",
                                   name=f"bc4_{b}_{h}")
                    nc.gpsimd.dma_start(
                        out=bch[:],
                        in_=bass.AP(
                            tensor=drec.tensor, offset=drec.offset,
                            ap=[[0, 64], [1, NIT * 512]],
                        ),
                    )
                    for it in range(NIT):
                        nc.vector.tensor_mul(
                            yh[g][po : po + 64, it * 512 : (it + 1) * 512],
                            hstgs[it][0:HD, :], bch[:, it, :],
                        )

                nc.leave_named_scope(f"attn{b}", sB[0], False)
                # ================= Phase C: output projection =================
                sC = nc.enter_named_scope(f"proj{b}", False)
                for tt2 in range(S // 128):
                    yo = outp.tile([128, D], f32, tag="yo")
                    for oc in range(2):
                        psow = psY.tile(
                            [128, 512], f32, tag="psy", name=f"pso_{b}_{tt2}_{oc}"
                        )
                        pso = psow[:, 0:512]
                        for kt in range(2):
                            nc.tensor.matmul(
                                pso,
                                yh[kt][:, tt2 * 128 : (tt2 + 1) * 128],
                                wo_sb[:, kt, oc * 512 : (oc + 1) * 512],
                                start=(kt == 0), stop=(kt == 1),
                            )
                        nc.vector.tensor_copy(yo[:, oc * 512 : (oc + 1) * 512], pso)
                    nc.sync.dma_start(
                        out=y.ap()[b, tt2 * 128 : (tt2 + 1) * 128, :], in_=yo[:]
                    )

                nc.leave_named_scope(f"proj{b}", sC[0], False)

    nc.compile()
    return nc


def _get_nc():
    global _NC
    if _NC is None:
        _NC = _build()
    return _NC


def kernel(x, w_qkv, b_qkv, w_proj, b_proj):
    global LAST_RESULT
    from concourse.bass_utils import run_bass_kernel_spmd

    x = np.asarray(x, dtype=np.float32)
    w_qkv = np.asarray(w_qkv, dtype=np.float32)
    b_qkv = np.asarray(b_qkv, dtype=np.float32)
    w_proj = np.asarray(w_proj, dtype=np.float32)
    b_proj = np.asarray(b_proj, dtype=np.float32)

    xTb = np.ascontiguousarray(x.transpose(0, 2, 1))  # [B, D, S]

    # causal masks for the 4 diagonal block offsets r: allow j'+128r <= i'
    jj = np.arange(128)[:, None]
    ii = np.arange(512)[None, :]
    mask = np.concatenate(
        [(jj + 128 * r <= ii).astype(np.float32) for r in range(4)], axis=1
    )  # [128, 2048]

    in_maps = []
    for c in range(8):
        bp, hg = c // 4, c % 4
        cols = slice(hg * HCOLS, (hg + 1) * HCOLS)
        w_q = w_qkv[:, cols]
        w_k = w_qkv[:, D : 2 * D][:, cols]
        w_v = w_qkv[:, 2 * D : 3 * D][:, cols]
        in_maps.append({
            "xT": np.ascontiguousarray(xTb[2 * bp : 2 * bp + 2]),
            "wqk": np.ascontiguousarray(np.concatenate([w_q, w_k], axis=1)),
            "wv": np.ascontiguousarray(w_v),
            "wo": np.ascontiguousarray(w_proj[cols, :]),
            "bqk": np.ascontiguousarray(
                np.concatenate([b_qkv[cols], b_qkv[D : 2 * D][cols]])
            ),
            "bv": np.ascontiguousarray(b_qkv[2 * D : 3 * D][cols]),
            "mask": mask,
        })

    nc = _get_nc()
    res = run_bass_kernel_spmd(nc, in_maps, core_ids=list(range(8)))
    LAST_RESULT = res

    out = np.zeros((B, S, D), dtype=np.float32)
    for c in range(8):
        bp = c // 4
        out[2 * bp : 2 * bp + 2] += res.results[c]["y"]
    out += b_proj[None, None, :]
    return out
